# revision 10
# baseline (speedup 1.0000x reference)
"""Trainium2 Bass kernel for nn_BenesBlock (quaternary Benes MLP-mixer block).

Strategy (v2):
  - Data parallel: 16 examples sharded 2-per-core across 8 NeuronCores.
  - Stream layout per example: j-blocked SBUF tile [96 part (u), 4096 free]
    with free index = j*1024 + l  (z = 4l + j in the Z-order sequence).
    A feature-major shadow copy S128 [128 part (f=j*96+u), 3 x 1024] is
    maintained by 6 SBUF->SBUF DMA pieces per layer (contiguous 4KB runs,
    nearly-free on the idle DMA engines).  This lets matmul1 contract over
    full K=128 tiles (3 x 6 x 1024 cols = 18432 PE cycles vs 24576 for the
    K=96 formulation).
  - matmul2 stays u-major (out chunks of 96 per j; K=768 already full):
    4j x 6k x 1024 cols = 24576 cycles.  Total PE 43008 cyc/ex-layer.
  - LayerNorm(axis=positions) via bn_stats/bn_aggr on DVE; inv_std via
    bit-trick + Newton (tiny [128,1] DVE ops are ~free); Gelu tanh on ACT
    with LN affine folded into per-partition scale/bias.
  - Residual: um = sigmoid(rs)*h + b2*CAND_W in ONE DVE tensor_scalar
    (2-scalar form), then combine = psum2 + um on GPSIMD writing through
    the permutation access pattern into the next j-blocked stream tile.
    (The b2 fold removes all ACT identity copies of the baseline.)
  - Permutations (qror/qrol/identity) are pure free-dim strided writes in
    the j-blocked layout (j stays in the free dim).
  - Z-order flatten/unflatten and all weight packing on host.
"""
import os
import sys
import numpy as np

for _p in ("/opt/trn_rl_repo", "/root/.axon_site/_ro/trn_rl_repo"):
    if os.path.isdir(_p) and _p not in sys.path:
        sys.path.insert(0, _p)

import concourse.bass as bass
import concourse.bacc as bacc
import concourse.mybir as mybir
import concourse.tile as tile
from concourse.bass_utils import run_bass_kernel_spmd

F32 = mybir.dt.float32
I32 = mybir.dt.int32
MMDT = mybir.dt.float32r   # dtype of all matmul operands
AF = mybir.ActivationFunctionType
ALU = mybir.AluOpType

N_CORES = 8
B, Wd, Ht, U = 16, 64, 64, 96
N = Wd * Ht                     # 4096 positions
BPC = B // N_CORES              # 2 examples per core
L = N // 4                      # 1024 groups
U4, U8 = 4 * U, 8 * U           # 384, 768
NC1 = U8 // 128                 # 6 v-chunks for matmul1 output
NK1 = U4 // 128                 # 3 k-tiles for matmul1 (f-major)
LN_EPS = 1e-3
RESIDUAL_W = 0.9
CAND_W = float(np.sqrt(1.0 - RESIDUAL_W**2) * 0.25)

# layer schedule: (unit index, permutation after the switch)
LAYERS = ([(0, 'ror')] * 5 + [(1, 'rol')] * 5 + [(2, 'mid')] +
          [(3, 'ror')] * 5 + [(4, 'rol')] * 5 + [(5, 'mid')])

# f-major repack pieces: (f0, n, j, u0) with f = j*96+u; chunk c = f0//128
REPACK = [(0, 96, 0, 0), (96, 32, 1, 0), (128, 64, 1, 32),
          (192, 64, 2, 0), (256, 32, 2, 64), (288, 96, 3, 0)]


def _z_order_flat_idx(w, h):
    n = w * h
    k = (w - 1).bit_length()
    z = np.arange(n)
    row = np.zeros(n, np.int64)
    col = np.zeros(n, np.int64)
    for b in range(k):
        q = (z >> (2 * b)) & 3
        row |= ((q >> 1) & 1) << b
        col |= (q & 1) << b
    return row * h + col


def build_bass():
    nc = bacc.Bacc("TRN2", target_bir_lowering=False, debug=False,
                   enable_asserts=False, num_devices=N_CORES)
    xs = nc.dram_tensor("xs", [BPC, 96, N], MMDT, kind="ExternalInput").ap()
    x128 = nc.dram_tensor("x128", [BPC, 128, NK1 * 1024], MMDT, kind="ExternalInput").ap()
    w1 = nc.dram_tensor("w1", [6, 128, NK1 * U8], MMDT, kind="ExternalInput").ap()
    w2 = nc.dram_tensor("w2", [6, 128, NC1 * U4], MMDT, kind="ExternalInput").ap()
    vg = nc.dram_tensor("vg", [96, 6 * 8], F32, kind="ExternalInput").ap()   # sig | b2c
    vl = nc.dram_tensor("vl", [128, 6 * 12 + 1], F32, kind="ExternalInput").ap()  # lnb | lnb^2+eps | rsqrt magic
    ys = nc.dram_tensor("ys", [BPC, 96, N], MMDT, kind="ExternalOutput").ap()

    with tile.TileContext(nc) as tc:
        with (
            tc.tile_pool(name="seqp", bufs=2) as seqp,
            tc.tile_pool(name="s128p", bufs=1) as s128p,
            tc.tile_pool(name="wp", bufs=2) as wp,
            tc.tile_pool(name="gp", bufs=1) as gp,
            tc.tile_pool(name="cp", bufs=1) as cp,
            tc.tile_pool(name="ump", bufs=4) as ump,
            tc.tile_pool(name="sp", bufs=8) as sp,
            tc.tile_pool(name="ps1p", bufs=5, space="PSUM") as ps1p,
            tc.tile_pool(name="ps2p", bufs=3, space="PSUM") as ps2p,
        ):
            # small per-unit constant vectors, loaded once (tiny, go first)
            vlt = cp.tile([128, 6 * 12 + 1], F32)
            nc.gpsimd.dma_start(vlt, vl)
            vgt = cp.tile([96, 6 * 8], F32)
            nc.gpsimd.dma_start(vgt, vg)

            # startup loads ordered for earliest mm1 start:
            # w1(unit0), x128 per k-tile per ex, w2(unit0), then j-blocked seqs
            w1t = wp.tile([128, NK1 * U8], MMDT, tag="w1", name="w1_0")
            nc.sync.dma_start(w1t, w1[0])
            seq, s128 = [], []
            for ex in range(BPC):
                t8 = s128p.tile([128, NK1 * 1024], MMDT, tag=f"s128_{ex}",
                                name=f"s128_{ex}_in")
                for k in range(NK1):
                    nc.sync.dma_start(t8[:, k * 1024:(k + 1) * 1024],
                                      x128[ex][:, k * 1024:(k + 1) * 1024])
                s128.append(t8)
            w2t = wp.tile([128, NC1 * U4], MMDT, tag="w2", name="w2_0")
            nc.sync.dma_start(w2t, w2[0])
            for ex in range(BPC):
                t = seqp.tile([96, N], MMDT, tag=f"seq{ex}", name=f"seq{ex}_in")
                nc.sync.dma_start(t, xs[ex])
                seq.append(t)

            cur_unit = [0]
            wts = {"w1": w1t, "w2": w2t}

            def load_weights(ui):
                if ui != cur_unit[0]:
                    cur_unit[0] = ui
                    w1n = wp.tile([128, NK1 * U8], MMDT, tag="w1", name=f"w1_{ui}")
                    nc.sync.dma_start(w1n, w1[ui])
                    w2n = wp.tile([128, NC1 * U4], MMDT, tag="w2", name=f"w2_{ui}")
                    nc.sync.dma_start(w2n, w2[ui])
                    wts["w1"], wts["w2"] = w1n, w2n
                return wts["w1"], wts["w2"]

            def phase1(li, ex, ui, w1t):
                """mm1 (f-major K=128) + LN + gelu -> g tile; returns g."""
                src128 = s128[ex]
                g = gp.tile([128, NC1 * 1024], MMDT, tag=f"g{ex}", name=f"g_{li}_{ex}")
                for c in range(NC1):
                    psh = []
                    st6 = sp.tile([128, 12], F32, tag="st6", name=f"st6_{li}_{ex}_{c}")
                    for h in range(2):
                        ps = ps1p.tile([128, 512], F32, tag="ps1", name=f"ps1_{li}_{ex}_{c}_{h}")
                        psh.append(ps)
                        for k in range(NK1):
                            lhs = w1t[:, k * U8 + c * 128: k * U8 + (c + 1) * 128]
                            rhs = src128[:, k * 1024 + 512 * h: k * 1024 + 512 * h + 512]
                            nc.tensor.matmul(ps, lhs, rhs,
                                             start=(k == 0), stop=(k == NK1 - 1))
                        # stats for this half as soon as its accumulation ends
                        nc.vector.bn_stats(st6[:, 6 * h: 6 * h + 6], ps)
                    mv = sp.tile([128, 2], F32, tag="mv", name=f"mv_{li}_{ex}_{c}")
                    nc.vector.bn_aggr(mv, st6)
                    # inv_std = (var + lnb^2 + eps) ** -0.5 ; bias = (lnb - mean)*inv_std
                    t0 = sp.tile([128, 1], F32, tag="t0", name=f"t0_{li}_{ex}_{c}")
                    nc.vector.tensor_add(t0, mv[:, 1:2], vlt[:, ui * 12 + 6 + c: ui * 12 + 7 + c])
                    sh = sp.tile([128, 1], F32, tag="sh", name=f"sh_{li}_{ex}_{c}")
                    nc.vector.tensor_scalar(sh.bitcast(I32), t0.bitcast(I32), 1, None,
                                            op0=ALU.arith_shift_right)
                    y0 = sp.tile([128, 1], F32, tag="y0", name=f"y0_{li}_{ex}_{c}")
                    nc.vector.tensor_tensor(y0.bitcast(I32), vlt[:, 72:73].bitcast(I32),
                                            sh.bitcast(I32), op=ALU.subtract)
                    kf = sp.tile([128, 1], F32, tag="kf", name=f"kf_{li}_{ex}_{c}")
                    nc.vector.tensor_scalar(kf, t0, -0.5, None, op0=ALU.mult)
                    yy = y0
                    for it in range(2):
                        aa = sp.tile([128, 1], F32, tag=f"aa{it}", name=f"aa{it}_{li}_{ex}_{c}")
                        nc.vector.tensor_mul(aa, yy, yy)
                        bb = sp.tile([128, 1], F32, tag=f"bb{it}", name=f"bb{it}_{li}_{ex}_{c}")
                        nc.vector.tensor_scalar(bb, aa, kf, 1.5,
                                                op0=ALU.mult, op1=ALU.add)
                        y2 = sp.tile([128, 1], F32, tag=f"y2{it}", name=f"y2{it}_{li}_{ex}_{c}")
                        nc.vector.tensor_mul(y2, yy, bb)
                        yy = y2
                    invs = yy
                    bia = sp.tile([128, 1], F32, tag="bia", name=f"bia_{li}_{ex}_{c}")
                    nc.vector.tensor_scalar(
                        bia, vlt[:, ui * 12 + c: ui * 12 + c + 1],
                        mv[:, 0:1], invs,
                        op0=ALU.subtract, op1=ALU.mult)
                    for h in range(2):
                        nc.scalar.activation(
                            g[:, c * 1024 + 512 * h: c * 1024 + 512 * h + 512],
                            psh[h], AF.Gelu_apprx_tanh, bias=bia, scale=invs)
                return g

            def phase2(li, ex, ui, perm, w2t, g, last):
                """mm2 + combine (permuted write) + f-major repack for next layer."""
                src = seq[ex]
                dst = seqp.tile([96, N], MMDT, tag=f"seq{ex}", name=f"seq{ex}_{li}")
                for j in range(4):
                    # um = sig_j * h_j   (DVE; b2c goes into the ACT bias)
                    um = ump.tile([96, 1024], F32, tag="um", name=f"um_{li}_{ex}_{j}")
                    nc.vector.tensor_scalar(
                        um, src[:, j * 1024: (j + 1) * 1024],
                        vgt[:, ui * 8 + j: ui * 8 + j + 1], None,
                        op0=ALU.mult)
                    for h in range(2):
                        ps2 = ps2p.tile([96, 512], F32, tag="ps2", name=f"ps2_{li}_{ex}_{j}_{h}")
                        for kc in range(NC1):
                            lhs2 = w2t[:, (kc * 4 + j) * 96: (kc * 4 + j + 1) * 96]
                            nc.tensor.matmul(
                                ps2, lhs2,
                                g[:, kc * 1024 + 512 * h: kc * 1024 + 512 * h + 512],
                                start=(kc == 0), stop=(kc == NC1 - 1))
                        # tmp = ps2 + b2c_j  (ACT affine, PSUM->SBUF)
                        tmp = ump.tile([96, 512], F32, tag="tmp", name=f"tmp_{li}_{ex}_{j}_{h}")
                        nc.scalar.activation(
                            tmp, ps2, AF.Identity,
                            bias=vgt[:, ui * 8 + 4 + j: ui * 8 + 5 + j])
                        umh = um[:, 512 * h: 512 * h + 512]
                        # combine = tmp + um on GPSIMD through the permutation AP
                        if perm == 'ror':
                            # dst free = (l>>8)*1024 + 4*(l&255) + j
                            dv = dst.rearrange("u (a t s) -> u a t s", a=4, s=4)[:, 2 * h: 2 * h + 2, :, j]
                            nc.gpsimd.tensor_add(
                                dv,
                                tmp.rearrange("u (a t) -> u a t", a=2),
                                umh.rearrange("u (a t) -> u a t", a=2))
                        elif perm == 'rol':
                            # dst free = jn*1024 + 256*j + t ; src l = 4t + jn
                            dv = dst.rearrange("u (s b) -> u s b", s=4)[:, :, 256 * j + 128 * h: 256 * j + 128 * h + 128]
                            nc.gpsimd.tensor_add(
                                dv,
                                tmp.rearrange("u (t s) -> u s t", s=4),
                                umh.rearrange("u (t s) -> u s t", s=4))
                        else:
                            nc.gpsimd.tensor_add(
                                dst[:, j * 1024 + 512 * h: j * 1024 + 512 * h + 512],
                                tmp, umh)
                            if last and h == 1:
                                # stream the finished j-block out immediately
                                nc.sync.dma_start(
                                    ys[ex][:, j * 1024: (j + 1) * 1024],
                                    dst[:, j * 1024: (j + 1) * 1024])
                seq[ex] = dst
                if not last:
                    # repack dst (j-blocked) -> f-major shadow for next mm1
                    t8 = s128p.tile([128, NK1 * 1024], MMDT, tag=f"s128_{ex}",
                                    name=f"s128_{ex}_{li}")
                    for (f0, n, j, u0) in REPACK:
                        c, p0 = divmod(f0, 128)
                        nc.sync.dma_start(
                            t8[p0:p0 + n, c * 1024:(c + 1) * 1024],
                            dst[u0:u0 + n, j * 1024:(j + 1) * 1024])
                    s128[ex] = t8

            # software pipeline across layers: P1(li,e0) is emitted right after
            # P2(li-1,e0), so its LN chain hides under P2(li-1,e1)'s matmuls.
            pend = None  # pending (li, ex, ui, perm, w2t, g, last) for phase2
            for li, (ui, perm) in enumerate(LAYERS):
                last = li == len(LAYERS) - 1
                w1c, w2c = load_weights(ui)
                g0 = phase1(li, 0, ui, w1c)
                if pend is not None:
                    phase2(*pend)
                g1 = phase1(li, 1, ui, w1c)
                phase2(li, 0, ui, perm, w2c, g0, last)
                pend = (li, 1, ui, perm, w2c, g1, last)
            phase2(*pend)
    if not nc.is_finalized():
        nc.finalize()
    return nc


_CACHED = {}


def _get_nc():
    if "nc" not in _CACHED:
        _CACHED["nc"] = build_bass()
    return _CACHED["nc"]


def _pack_inputs(x, W1, ln_bias, W2, b2, res_scale):
    x = np.ascontiguousarray(np.asarray(x, np.float32))
    W1 = np.asarray(W1, np.float32)
    W2 = np.asarray(W2, np.float32)
    b2 = np.asarray(b2, np.float32)
    ln_bias = np.asarray(ln_bias, np.float32)
    res_scale = np.asarray(res_scale, np.float32)

    flat = _z_order_flat_idx(Wd, Ht)
    seq_z = x.reshape(B, N, U)[:, flat]                      # [B, 4096, 96]
    # j-blocked: xs[b, u, j*1024 + l] = seq_z[b, 4l+j, u]
    xs_jb = np.ascontiguousarray(
        seq_z.reshape(B, L, 4, U).transpose(0, 3, 2, 1)      # (b, u, j, l)
        .reshape(B, U, N))
    # f-major: x128[b, p, c*1024 + l] = h[l, f=128c+p]; h[l, f] = seq_z[b, 4l+f//96, f%96]
    hT = seq_z.reshape(B, L, U4).transpose(0, 2, 1)          # [B, 384, 1024]
    x128 = np.ascontiguousarray(
        hT.reshape(B, NK1, 128, L).transpose(0, 2, 1, 3).reshape(B, 128, NK1 * L))

    w1p = np.ascontiguousarray(
        W1.reshape(6, U4, U8).reshape(6, NK1, 128, U8)
        .transpose(0, 2, 1, 3).reshape(6, 128, NK1 * U8))
    w2p = np.ascontiguousarray(
        (W2.reshape(6, U8, U4) * CAND_W).reshape(6, NC1, 128, 4, 96)
        .transpose(0, 2, 1, 3, 4).reshape(6, 128, NC1 * U4))
    sig = np.stack([(1.0 / (1.0 + np.exp(-res_scale.reshape(6, U4)[k]))).reshape(4, 96).T
                    for k in range(6)])
    b2c = np.stack([(CAND_W * b2.reshape(6, U4)[k]).reshape(4, 96).T
                    for k in range(6)])
    vgp = np.ascontiguousarray(
        np.concatenate([sig, b2c], axis=2).transpose(1, 0, 2).reshape(96, 48))
    lnbp = np.stack([ln_bias.reshape(6, U8)[k].reshape(NC1, 128).T for k in range(6)])
    vlp = np.concatenate([lnbp, lnbp**2 + LN_EPS], axis=2).transpose(1, 0, 2).reshape(128, 72)
    magic = np.full((128, 1), np.uint32(0x5f3759df), np.uint32).view(np.float32)
    vlp = np.ascontiguousarray(np.concatenate([vlp, magic], axis=1))
    return xs_jb, x128, w1p, w2p, vgp, vlp


def kernel(x, W1, ln_bias, W2, b2, res_scale, _trace=False, _tmpdir=None):
    xs_jb, x128, w1p, w2p, vgp, vlp = _pack_inputs(x, W1, ln_bias, W2, b2, res_scale)
    nc = _get_nc()
    in_maps = []
    for core in range(N_CORES):
        in_maps.append({
            "xs": np.ascontiguousarray(xs_jb[core * BPC:(core + 1) * BPC]),
            "x128": np.ascontiguousarray(x128[core * BPC:(core + 1) * BPC]),
            "w1": w1p, "w2": w2p, "vg": vgp, "vl": vlp,
        })
    res = run_bass_kernel_spmd(nc, in_maps, core_ids=list(range(N_CORES)),
                               trace=_trace, tmpdir=_tmpdir,
                               stitch_traces=False)
    outT = np.concatenate([res.results[c]["ys"] for c in range(N_CORES)], axis=0)

    # outT: [B, 96, 4096] j-blocked -> seq_z order -> inverse z-order
    flat = _z_order_flat_idx(Wd, Ht)
    inv = np.argsort(flat)
    seq_z = outT.reshape(B, U, 4, L).transpose(0, 3, 2, 1).reshape(B, N, U)
    out = seq_z[:, inv].reshape(B, Wd, Ht, U)
    if _trace:
        return np.ascontiguousarray(out.astype(np.float32)), res
    return np.ascontiguousarray(out.astype(np.float32))


# revision 11
# speedup vs baseline: 1.0641x; 1.0641x over previous
"""Trainium2 Bass kernel for nn_BenesBlock (quaternary Benes MLP-mixer block).

Strategy (v2):
  - Data parallel: 16 examples sharded 2-per-core across 8 NeuronCores.
  - Stream layout per example: j-blocked SBUF tile [96 part (u), 4096 free]
    with free index = j*1024 + l  (z = 4l + j in the Z-order sequence).
    A feature-major shadow copy S128 [128 part (f=j*96+u), 3 x 1024] is
    maintained by 6 SBUF->SBUF DMA pieces per layer (contiguous 4KB runs,
    nearly-free on the idle DMA engines).  This lets matmul1 contract over
    full K=128 tiles (3 x 6 x 1024 cols = 18432 PE cycles vs 24576 for the
    K=96 formulation).
  - matmul2 stays u-major (out chunks of 96 per j; K=768 already full):
    4j x 6k x 1024 cols = 24576 cycles.  Total PE 43008 cyc/ex-layer.
  - LayerNorm(axis=positions) via bn_stats/bn_aggr on DVE; inv_std via
    bit-trick + Newton (tiny [128,1] DVE ops are ~free); Gelu tanh on ACT
    with LN affine folded into per-partition scale/bias.
  - Residual: um = sigmoid(rs)*h + b2*CAND_W in ONE DVE tensor_scalar
    (2-scalar form), then combine = psum2 + um on GPSIMD writing through
    the permutation access pattern into the next j-blocked stream tile.
    (The b2 fold removes all ACT identity copies of the baseline.)
  - Permutations (qror/qrol/identity) are pure free-dim strided writes in
    the j-blocked layout (j stays in the free dim).
  - Z-order flatten/unflatten and all weight packing on host.
"""
import os
import sys
import numpy as np

for _p in ("/opt/trn_rl_repo", "/root/.axon_site/_ro/trn_rl_repo"):
    if os.path.isdir(_p) and _p not in sys.path:
        sys.path.insert(0, _p)

import concourse.bass as bass
import concourse.bacc as bacc
import concourse.mybir as mybir
import concourse.tile as tile
from concourse.bass_utils import run_bass_kernel_spmd

F32 = mybir.dt.float32
I32 = mybir.dt.int32
MMDT = mybir.dt.float32r   # dtype of all matmul operands
AF = mybir.ActivationFunctionType
ALU = mybir.AluOpType

N_CORES = 8
B, Wd, Ht, U = 16, 64, 64, 96
N = Wd * Ht                     # 4096 positions
BPC = B // N_CORES              # 2 examples per core
L = N // 4                      # 1024 groups
U4, U8 = 4 * U, 8 * U           # 384, 768
NC1 = U8 // 128                 # 6 v-chunks for matmul1 output
NK1 = U4 // 128                 # 3 k-tiles for matmul1 (f-major)
LN_EPS = 1e-3
RESIDUAL_W = 0.9
CAND_W = float(np.sqrt(1.0 - RESIDUAL_W**2) * 0.25)

# layer schedule: (unit index, permutation after the switch)
LAYERS = ([(0, 'ror')] * 5 + [(1, 'rol')] * 5 + [(2, 'mid')] +
          [(3, 'ror')] * 5 + [(4, 'rol')] * 5 + [(5, 'mid')])

# f-major repack pieces: (f0, n, j, u0) with f = j*96+u; chunk c = f0//128
REPACK = [(0, 96, 0, 0), (96, 32, 1, 0), (128, 64, 1, 32),
          (192, 64, 2, 0), (256, 32, 2, 64), (288, 96, 3, 0)]


def _z_order_flat_idx(w, h):
    n = w * h
    k = (w - 1).bit_length()
    z = np.arange(n)
    row = np.zeros(n, np.int64)
    col = np.zeros(n, np.int64)
    for b in range(k):
        q = (z >> (2 * b)) & 3
        row |= ((q >> 1) & 1) << b
        col |= (q & 1) << b
    return row * h + col


def build_bass():
    nc = bacc.Bacc("TRN2", target_bir_lowering=False, debug=False,
                   enable_asserts=False, num_devices=N_CORES)
    xs = nc.dram_tensor("xs", [BPC, 96, N], MMDT, kind="ExternalInput").ap()
    x128 = nc.dram_tensor("x128", [BPC, 128, NK1 * 1024], MMDT, kind="ExternalInput").ap()
    w1 = nc.dram_tensor("w1", [6, 128, NK1 * U8], MMDT, kind="ExternalInput").ap()
    w2 = nc.dram_tensor("w2", [6, 128, NC1 * U4], MMDT, kind="ExternalInput").ap()
    vg = nc.dram_tensor("vg", [96, 6 * 8], F32, kind="ExternalInput").ap()   # sig | b2c
    vl = nc.dram_tensor("vl", [128, 6 * 12 + 1], F32, kind="ExternalInput").ap()  # lnb | lnb^2+eps | rsqrt magic
    ys = nc.dram_tensor("ys", [BPC, 96, N], MMDT, kind="ExternalOutput").ap()

    with tile.TileContext(nc) as tc:
        with (
            tc.tile_pool(name="seqp", bufs=2) as seqp,
            tc.tile_pool(name="s128p", bufs=1) as s128p,
            tc.tile_pool(name="wp", bufs=2) as wp,
            tc.tile_pool(name="gp", bufs=1) as gp,
            tc.tile_pool(name="cp", bufs=1) as cp,
            tc.tile_pool(name="ump", bufs=4) as ump,
            tc.tile_pool(name="sp", bufs=8) as sp,
            tc.tile_pool(name="ps1p", bufs=5, space="PSUM") as ps1p,
            tc.tile_pool(name="ps2p", bufs=3, space="PSUM") as ps2p,
        ):
            # small per-unit constant vectors, loaded once (tiny, go first)
            vlt = cp.tile([128, 6 * 12 + 1], F32)
            nc.gpsimd.dma_start(vlt, vl)
            vgt = cp.tile([96, 6 * 8], F32)
            nc.gpsimd.dma_start(vgt, vg)

            # startup loads ordered for earliest mm1 start:
            # w1(unit0), x128 per k-tile per ex, w2(unit0), then j-blocked seqs
            w1t = wp.tile([128, NK1 * U8], MMDT, tag="w1", name="w1_0")
            nc.sync.dma_start(w1t, w1[0])
            seq, s128 = [], []
            for ex in range(BPC):
                t8 = s128p.tile([128, NK1 * 1024], MMDT, tag=f"s128_{ex}",
                                name=f"s128_{ex}_in")
                for k in range(NK1):
                    nc.sync.dma_start(t8[:, k * 1024:(k + 1) * 1024],
                                      x128[ex][:, k * 1024:(k + 1) * 1024])
                s128.append(t8)
            w2t = wp.tile([128, NC1 * U4], MMDT, tag="w2", name="w2_0")
            nc.sync.dma_start(w2t, w2[0])
            for ex in range(BPC):
                t = seqp.tile([96, N], MMDT, tag=f"seq{ex}", name=f"seq{ex}_in")
                nc.sync.dma_start(t, xs[ex])
                seq.append(t)

            cur_unit = [0]
            wts = {"w1": w1t, "w2": w2t}

            def load_weights(ui):
                if ui != cur_unit[0]:
                    cur_unit[0] = ui
                    w1n = wp.tile([128, NK1 * U8], MMDT, tag="w1", name=f"w1_{ui}")
                    nc.sync.dma_start(w1n, w1[ui])
                    w2n = wp.tile([128, NC1 * U4], MMDT, tag="w2", name=f"w2_{ui}")
                    nc.sync.dma_start(w2n, w2[ui])
                    wts["w1"], wts["w2"] = w1n, w2n
                return wts["w1"], wts["w2"]

            def phase1_chunk(li, ex, ui, w1t, g, c):
                """one mm1 v-chunk (f-major K=128) + LN + gelu into g."""
                src128 = s128[ex]
                if True:
                    psh = []
                    st6 = sp.tile([128, 12], F32, tag="st6", name=f"st6_{li}_{ex}_{c}")
                    for h in range(2):
                        ps = ps1p.tile([128, 512], F32, tag="ps1", name=f"ps1_{li}_{ex}_{c}_{h}")
                        psh.append(ps)
                        for k in range(NK1):
                            lhs = w1t[:, k * U8 + c * 128: k * U8 + (c + 1) * 128]
                            rhs = src128[:, k * 1024 + 512 * h: k * 1024 + 512 * h + 512]
                            nc.tensor.matmul(ps, lhs, rhs,
                                             start=(k == 0), stop=(k == NK1 - 1))
                        # stats for this half as soon as its accumulation ends
                        nc.vector.bn_stats(st6[:, 6 * h: 6 * h + 6], ps)
                    mv = sp.tile([128, 2], F32, tag="mv", name=f"mv_{li}_{ex}_{c}")
                    nc.vector.bn_aggr(mv, st6)
                    # inv_std = (var + lnb^2 + eps) ** -0.5 ; bias = (lnb - mean)*inv_std
                    t0 = sp.tile([128, 1], F32, tag="t0", name=f"t0_{li}_{ex}_{c}")
                    nc.vector.tensor_add(t0, mv[:, 1:2], vlt[:, ui * 12 + 6 + c: ui * 12 + 7 + c])
                    sh = sp.tile([128, 1], F32, tag="sh", name=f"sh_{li}_{ex}_{c}")
                    nc.vector.tensor_scalar(sh.bitcast(I32), t0.bitcast(I32), 1, None,
                                            op0=ALU.arith_shift_right)
                    y0 = sp.tile([128, 1], F32, tag="y0", name=f"y0_{li}_{ex}_{c}")
                    nc.vector.tensor_tensor(y0.bitcast(I32), vlt[:, 72:73].bitcast(I32),
                                            sh.bitcast(I32), op=ALU.subtract)
                    kf = sp.tile([128, 1], F32, tag="kf", name=f"kf_{li}_{ex}_{c}")
                    nc.vector.tensor_scalar(kf, t0, -0.5, None, op0=ALU.mult)
                    yy = y0
                    for it in range(2):
                        aa = sp.tile([128, 1], F32, tag=f"aa{it}", name=f"aa{it}_{li}_{ex}_{c}")
                        nc.vector.tensor_mul(aa, yy, yy)
                        bb = sp.tile([128, 1], F32, tag=f"bb{it}", name=f"bb{it}_{li}_{ex}_{c}")
                        nc.vector.tensor_scalar(bb, aa, kf, 1.5,
                                                op0=ALU.mult, op1=ALU.add)
                        y2 = sp.tile([128, 1], F32, tag=f"y2{it}", name=f"y2{it}_{li}_{ex}_{c}")
                        nc.vector.tensor_mul(y2, yy, bb)
                        yy = y2
                    invs = yy
                    bia = sp.tile([128, 1], F32, tag="bia", name=f"bia_{li}_{ex}_{c}")
                    nc.vector.tensor_scalar(
                        bia, vlt[:, ui * 12 + c: ui * 12 + c + 1],
                        mv[:, 0:1], invs,
                        op0=ALU.subtract, op1=ALU.mult)
                    for h in range(2):
                        nc.scalar.activation(
                            g[:, c * 1024 + 512 * h: c * 1024 + 512 * h + 512],
                            psh[h], AF.Gelu_apprx_tanh, bias=bia, scale=invs)

            def phase2_j(li, ex, ui, perm, w2t, g, last, src, dst, j):
                """one mm2 j-block + combine through the permutation AP."""
                if True:
                    # um = sig_j * h_j   (DVE; b2c goes into the ACT bias)
                    um = ump.tile([96, 1024], F32, tag="um", name=f"um_{li}_{ex}_{j}")
                    nc.vector.tensor_scalar(
                        um, src[:, j * 1024: (j + 1) * 1024],
                        vgt[:, ui * 8 + j: ui * 8 + j + 1], None,
                        op0=ALU.mult)
                    for h in range(2):
                        ps2 = ps2p.tile([96, 512], F32, tag="ps2", name=f"ps2_{li}_{ex}_{j}_{h}")
                        for kc in range(NC1):
                            lhs2 = w2t[:, (kc * 4 + j) * 96: (kc * 4 + j + 1) * 96]
                            nc.tensor.matmul(
                                ps2, lhs2,
                                g[:, kc * 1024 + 512 * h: kc * 1024 + 512 * h + 512],
                                start=(kc == 0), stop=(kc == NC1 - 1))
                        # tmp = ps2 + b2c_j  (ACT affine, PSUM->SBUF)
                        tmp = ump.tile([96, 512], F32, tag="tmp", name=f"tmp_{li}_{ex}_{j}_{h}")
                        nc.scalar.activation(
                            tmp, ps2, AF.Identity,
                            bias=vgt[:, ui * 8 + 4 + j: ui * 8 + 5 + j])
                        umh = um[:, 512 * h: 512 * h + 512]
                        # combine = tmp + um on GPSIMD through the permutation AP
                        if perm == 'ror':
                            # dst free = (l>>8)*1024 + 4*(l&255) + j
                            dv = dst.rearrange("u (a t s) -> u a t s", a=4, s=4)[:, 2 * h: 2 * h + 2, :, j]
                            nc.gpsimd.tensor_add(
                                dv,
                                tmp.rearrange("u (a t) -> u a t", a=2),
                                umh.rearrange("u (a t) -> u a t", a=2))
                        elif perm == 'rol':
                            # dst free = jn*1024 + 256*j + t ; src l = 4t + jn
                            dv = dst.rearrange("u (s b) -> u s b", s=4)[:, :, 256 * j + 128 * h: 256 * j + 128 * h + 128]
                            nc.gpsimd.tensor_add(
                                dv,
                                tmp.rearrange("u (t s) -> u s t", s=4),
                                umh.rearrange("u (t s) -> u s t", s=4))
                        else:
                            nc.gpsimd.tensor_add(
                                dst[:, j * 1024 + 512 * h: j * 1024 + 512 * h + 512],
                                tmp, umh)
                            if last and h == 1:
                                # stream the finished j-block out immediately
                                nc.sync.dma_start(
                                    ys[ex][:, j * 1024: (j + 1) * 1024],
                                    dst[:, j * 1024: (j + 1) * 1024])

            # Software pipeline: engines execute in program order, so the
            # cover work must be EMITTED between the stalling units.  Per layer
            # we interleave mm1 chunks (phase1, one example) with mm2 j-blocks
            # (phase2, the other example / previous layer) at unit granularity.
            def p1_units(li, ex, ui, w1t):
                g = gp.tile([128, NC1 * 1024], MMDT, tag=f"g{ex}", name=f"g_{li}_{ex}")
                for c in range(NC1):
                    phase1_chunk(li, ex, ui, w1t, g, c)
                    yield
                yield g

            def p2_units(li, ex, ui, perm, w2t, g, last):
                src = seq[ex]
                dst = seqp.tile([96, N], MMDT, tag=f"seq{ex}", name=f"seq{ex}_{li}")
                for j in range(4):
                    phase2_j(li, ex, ui, perm, w2t, g, last, src, dst, j)
                    yield
                seq[ex] = dst
                if not last:
                    # repack dst (j-blocked) -> f-major shadow for next mm1
                    t8 = s128p.tile([128, NK1 * 1024], MMDT, tag=f"s128_{ex}",
                                    name=f"s128_{ex}_{li}")
                    for (f0, n, j, u0) in REPACK:
                        c, p0 = divmod(f0, 128)
                        nc.sync.dma_start(
                            t8[p0:p0 + n, c * 1024:(c + 1) * 1024],
                            dst[u0:u0 + n, j * 1024:(j + 1) * 1024])
                    s128[ex] = t8
                yield

            def interleave(a_gen, b_gen):
                """Emit a0 b0 a1 b1 ... then remaining a's; returns a's value."""
                ret = None
                while True:
                    try:
                        v = next(a_gen)
                        if v is not None:
                            ret = v
                    except StopIteration:
                        break
                    if b_gen is not None:
                        try:
                            next(b_gen)
                        except StopIteration:
                            b_gen = None
                while b_gen is not None:
                    try:
                        next(b_gen)
                    except StopIteration:
                        break
                return ret

            pend = None  # P2 generator for (li-1, ex1)
            for li, (ui, perm) in enumerate(LAYERS):
                last = li == len(LAYERS) - 1
                w1c, w2c = load_weights(ui)
                g0 = interleave(p1_units(li, 0, ui, w1c), pend)
                b0 = p2_units(li, 0, ui, perm, w2c, g0, last)
                g1 = interleave(p1_units(li, 1, ui, w1c), b0)
                pend = p2_units(li, 1, ui, perm, w2c, g1, last)
            while True:
                try:
                    next(pend)
                except StopIteration:
                    break
    if not nc.is_finalized():
        nc.finalize()
    return nc


_CACHED = {}


def _get_nc():
    if "nc" not in _CACHED:
        _CACHED["nc"] = build_bass()
    return _CACHED["nc"]


def _pack_inputs(x, W1, ln_bias, W2, b2, res_scale):
    x = np.ascontiguousarray(np.asarray(x, np.float32))
    W1 = np.asarray(W1, np.float32)
    W2 = np.asarray(W2, np.float32)
    b2 = np.asarray(b2, np.float32)
    ln_bias = np.asarray(ln_bias, np.float32)
    res_scale = np.asarray(res_scale, np.float32)

    flat = _z_order_flat_idx(Wd, Ht)
    seq_z = x.reshape(B, N, U)[:, flat]                      # [B, 4096, 96]
    # j-blocked: xs[b, u, j*1024 + l] = seq_z[b, 4l+j, u]
    xs_jb = np.ascontiguousarray(
        seq_z.reshape(B, L, 4, U).transpose(0, 3, 2, 1)      # (b, u, j, l)
        .reshape(B, U, N))
    # f-major: x128[b, p, c*1024 + l] = h[l, f=128c+p]; h[l, f] = seq_z[b, 4l+f//96, f%96]
    hT = seq_z.reshape(B, L, U4).transpose(0, 2, 1)          # [B, 384, 1024]
    x128 = np.ascontiguousarray(
        hT.reshape(B, NK1, 128, L).transpose(0, 2, 1, 3).reshape(B, 128, NK1 * L))

    w1p = np.ascontiguousarray(
        W1.reshape(6, U4, U8).reshape(6, NK1, 128, U8)
        .transpose(0, 2, 1, 3).reshape(6, 128, NK1 * U8))
    w2p = np.ascontiguousarray(
        (W2.reshape(6, U8, U4) * CAND_W).reshape(6, NC1, 128, 4, 96)
        .transpose(0, 2, 1, 3, 4).reshape(6, 128, NC1 * U4))
    sig = np.stack([(1.0 / (1.0 + np.exp(-res_scale.reshape(6, U4)[k]))).reshape(4, 96).T
                    for k in range(6)])
    b2c = np.stack([(CAND_W * b2.reshape(6, U4)[k]).reshape(4, 96).T
                    for k in range(6)])
    vgp = np.ascontiguousarray(
        np.concatenate([sig, b2c], axis=2).transpose(1, 0, 2).reshape(96, 48))
    lnbp = np.stack([ln_bias.reshape(6, U8)[k].reshape(NC1, 128).T for k in range(6)])
    vlp = np.concatenate([lnbp, lnbp**2 + LN_EPS], axis=2).transpose(1, 0, 2).reshape(128, 72)
    magic = np.full((128, 1), np.uint32(0x5f3759df), np.uint32).view(np.float32)
    vlp = np.ascontiguousarray(np.concatenate([vlp, magic], axis=1))
    return xs_jb, x128, w1p, w2p, vgp, vlp


def kernel(x, W1, ln_bias, W2, b2, res_scale, _trace=False, _tmpdir=None):
    xs_jb, x128, w1p, w2p, vgp, vlp = _pack_inputs(x, W1, ln_bias, W2, b2, res_scale)
    nc = _get_nc()
    in_maps = []
    for core in range(N_CORES):
        in_maps.append({
            "xs": np.ascontiguousarray(xs_jb[core * BPC:(core + 1) * BPC]),
            "x128": np.ascontiguousarray(x128[core * BPC:(core + 1) * BPC]),
            "w1": w1p, "w2": w2p, "vg": vgp, "vl": vlp,
        })
    res = run_bass_kernel_spmd(nc, in_maps, core_ids=list(range(N_CORES)),
                               trace=_trace, tmpdir=_tmpdir,
                               stitch_traces=False)
    outT = np.concatenate([res.results[c]["ys"] for c in range(N_CORES)], axis=0)

    # outT: [B, 96, 4096] j-blocked -> seq_z order -> inverse z-order
    flat = _z_order_flat_idx(Wd, Ht)
    inv = np.argsort(flat)
    seq_z = outT.reshape(B, U, 4, L).transpose(0, 3, 2, 1).reshape(B, N, U)
    out = seq_z[:, inv].reshape(B, Wd, Ht, U)
    if _trace:
        return np.ascontiguousarray(out.astype(np.float32)), res
    return np.ascontiguousarray(out.astype(np.float32))
